# revision 1
# baseline (speedup 1.0000x reference)
"""Trainium2 Bass kernel for nn_Attention_46222438039802.

Reference computation:
    Q      = inputs @ WQ                    # (B,S,F)
    Kmat   = label_emb @ WK                 # (C,F)
    scores = Q @ Kmat^T                     # (B,S,C)
    A      = softmax(scores, axis=-1)
    V      = label_emb @ WV                 # (C,F)
    out    = A @ V                          # (B,S,F)

Key algebraic rewrite: Q is only ever used through `scores`, so
    scores = inputs @ (WQ @ Kmat^T) = inputs @ P,   P : (F, C)
The (B*S, F) @ (F, F) Q-projection (34 GFLOP) collapses into a host-side
weight-folding producing P (F x C) and V (C x F).  The device computes
    out = softmax(inputs @ P) @ V
data-parallel over the batch dim (1 batch element per NeuronCore).

Device layout choices (per core, x = inputs[b], pre-transposed on host):
  - xT (F, S) so the contraction dim F lies on SBUF partitions.
  - scoresT = P^T-chunks @ xT-chunks accumulated in PSUM as [C=64, S] --
    P-chunk is the stationary operand, so the PE streams 512-wide.
  - exp on the Scalar engine straight out of PSUM (softmax max-subtraction
    skipped: scores are ~N(0,1), |s| < ~7, exp is safe in fp32).
  - expT [64, S] is *already* the stationary-operand layout for A @ V:
    out_tile [128s, F] = expT_tile^T @ V.  The softmax denominator comes
    from a ones-column appended to V on the host (V_aug[:, F] == 1), via a
    matmul reusing the same stationary weights.  Zero transposes anywhere.
  - softmax normalization fused into the mandatory PSUM->SBUF copy
    (Copy-activation with per-partition scale = 1/denom), split across the
    Scalar and Vector engines.

Built as bacc.Bacc and legalized with nc.compile(): TRN2 instructions may
carry at most one semaphore wait, and Bacc's generate_event_semaphores
pass splits anything wider.
"""

import numpy as np

import concourse.bass as bass
import concourse.mybir as mybir
from concourse import bacc, bass_utils
from concourse.tile import TileContext

B, S, F, C = 8, 2048, 1024, 64
N_CORES = 8
FP32 = mybir.dt.float32
FP32R = mybir.dt.float32r

KC = F // 128            # 8 contraction chunks of 128
N_HALF = 2               # process S in halves to fit PSUM
SH = S // N_HALF         # 1024 rows per half
NT = SH // 128           # 8 output s-tiles per half


def _build_bass(n_iters: int = 1, variant: str = "bigstore",
                n_blocks: int = 4) -> bass.Bass:
    """Build the kernel; n_iters > 1 wraps the computation in a hardware
    For_i loop for wall-clock slope benchmarking (kernel() uses n_iters=1).
    variant: 'full' | 'dma_only' (loads + stores, no compute) |
    'bigstore' (one store per S-block).  n_blocks: S-block pipelining
    granularity (2 or 4)."""
    nc = bacc.Bacc()
    NB = n_blocks
    SB = S // NB             # rows per block
    NTB = SB // 128          # output s-tiles per block

    xT = nc.dram_tensor("xT", [F, S], FP32R, kind="ExternalInput")
    Pr = nc.dram_tensor("Pr", [128, KC * C], FP32R, kind="ExternalInput")
    Vm = nc.dram_tensor("Vm", [C, F + 1], FP32R, kind="ExternalInput")
    out = nc.dram_tensor("out", [S, F], FP32, kind="ExternalOutput")

    with TileContext(nc) as tc:
        with (
            tc.tile_pool(name="consts", bufs=1) as consts,
            tc.tile_pool(name="xt", bufs=1) as xt_pool,
            tc.tile_pool(name="expT", bufs=2) as exp_pool,
            tc.tile_pool(name="recip", bufs=2) as recip_pool,
            tc.tile_pool(name="osb", bufs=3 if variant in ("bigstore", "nostore", "halfstore") else 4) as out_pool,
            tc.tile_pool(name="scps", bufs=(2 if n_blocks >= 4 else 1), space="PSUM") as sc_psum,
            tc.tile_pool(name="numps", bufs=2, space="PSUM") as num_psum,
            tc.tile_pool(name="denps", bufs=2, space="PSUM") as den_psum,
        ):
          def one_iter(_iv=None):
              # First activation block DMA goes out ahead of the small const
              # loads so block-0 compute starts as early as possible.
              xt_all = xt_pool.tile([128, KC * S], FP32R, tag="xt")

              def load_block(hh):
                  nc.sync.dma_start(
                      xt_all[:, :].rearrange(
                          "p (k hh s) -> p k hh s", k=KC, hh=NB
                      )[:, :, hh, :],
                      xT[:, hh * SB : (hh + 1) * SB].rearrange(
                          "(k p) s -> p k s", p=128
                      ),
                  )

              load_block(0)
              P_sb = consts.tile([128, KC * C], FP32R)
              nc.sync.dma_start(P_sb[:], Pr[:, :])
              V_sb = consts.tile([C, F + 1], FP32R)
              nc.sync.dma_start(V_sb[:], Vm[:, :])
              for hh in range(1, NB):
                  load_block(hh)
              xts = [xt_all[:, k * S : (k + 1) * S] for k in range(KC)]

              if variant == "dma_only":
                  for h in range(NB):
                      dst = out[h * SB : (h + 1) * SB, :].rearrange(
                          "(t p) f -> p t f", p=128
                      )
                      srcv = xt_all[:, h * NTB * F : (h + 1) * NTB * F].rearrange(
                          "p (t f) -> p t f", f=F
                      )
                      nc.sync.dma_start(dst, srcv)
                  return

              osb_list = []
              for h in range(NB):
                  # scoresT[c, s] for this half, accumulated over the F dim.
                  scT = sc_psum.tile([C, SB], FP32)
                  for k in range(KC):
                      for n in range(SB // 512):
                          nc.tensor.matmul(
                              scT[:, n * 512 : (n + 1) * 512],
                              lhsT=P_sb[:, k * C : (k + 1) * C],
                              rhs=xts[k][:, h * SB + n * 512 : h * SB + (n + 1) * 512],
                              start=(k == 0),
                              stop=(k == KC - 1),
                          )

                  expT = exp_pool.tile([C, SB], FP32R)
                  nc.scalar.activation(
                      expT[:], scT[:], mybir.ActivationFunctionType.Exp
                  )
                  if variant == "phase1only":
                      continue

                  # Row-sums of exp via the ones-column of V_aug.
                  den = den_psum.tile([128, NTB], FP32)
                  for t in range(NTB):
                      # N=1 fails the fp32r ISA check; run this one in plain
                      # fp32 (same bits) via bitcast.
                      nc.tensor.matmul(
                          den[:, t : t + 1],
                          lhsT=expT[:, t * 128 : (t + 1) * 128].bitcast(FP32),
                          rhs=V_sb[:, F : F + 1].bitcast(FP32),
                          start=True,
                          stop=True,
                      )
                  recip = recip_pool.tile([128, NTB], FP32)
                  nc.vector.reciprocal(recip[:], den[:])

                  for t in range(NTB):
                      num = num_psum.tile([128, F], FP32)
                      for n in range(F // 512):
                          nc.tensor.matmul(
                              num[:, n * 512 : (n + 1) * 512],
                              lhsT=expT[:, t * 128 : (t + 1) * 128],
                              rhs=V_sb[:, n * 512 : (n + 1) * 512],
                              start=True,
                              stop=True,
                          )
                      if variant in ("bigstore", "nostore", "halfstore"):
                          if t == 0:
                              osb_big = out_pool.tile([128, NTB * F], FP32,
                                                      tag="osb")
                          osb = osb_big[:, t * F : (t + 1) * F]
                      else:
                          osb = out_pool.tile([128, F], FP32, tag="osb")
                      # Normalize while copying PSUM->SBUF, split across the
                      # Scalar and Vector engines.
                      nc.scalar.mul(osb[:, 0:512], num[:, 0:512], recip[:, t : t + 1])
                      nc.vector.tensor_scalar_mul(
                          osb[:, 512:1024], num[:, 512:1024], recip[:, t : t + 1]
                      )
                      if variant in ("bigstore", "nostore", "halfstore"):
                          if variant == "halfstore" and t % 2 == 1:
                              # Store each 2-tile (1 MiB) group as soon as it
                              # is normalized: earlier stores widen the
                              # read/write DMA overlap window.
                              row0 = h * SB + (t - 1) * 128
                              dst = out[row0 : row0 + 256, :].rearrange(
                                  "(t p) f -> p t f", p=128
                              )
                              srcv = osb_big[:, (t - 1) * F : (t + 1) * F].rearrange(
                                  "p (t f) -> p t f", f=F
                              )
                              nc.scalar.dma_start(dst, srcv)
                          elif t == NTB - 1 and variant == "bigstore":
                              row0 = h * SB
                              dst = out[row0 : row0 + SB, :].rearrange(
                                  "(t p) f -> p t f", p=128
                              )
                              srcv = osb_big[:, :].rearrange(
                                  "p (t f) -> p t f", f=F
                              )
                              # Store on the Scalar engine's HWDGE ring so
                              # stores overlap the SP-ring input loads.
                              # (Measured: beats SWDGE/gpsimd stores by ~6us
                              # and sync-ring stores by ~10us.)
                              nc.scalar.dma_start(dst, srcv)
                      else:
                          row0 = h * SB + t * 128
                          nc.sync.dma_start(out[row0 : row0 + 128, :], osb[:])

          if n_iters == 1:
              one_iter()
          else:
              with tc.For_i(0, n_iters, 1) as iv:
                  one_iter(iv)

    nc.compile()
    return nc


_NC_CACHE: list = []


def _get_nc() -> bass.Bass:
    if not _NC_CACHE:
        _NC_CACHE.append(_build_bass())
    return _NC_CACHE[0]


def _prep_weights(WQ, label_emb, WK, WV):
    Kmat = label_emb @ WK                 # (C, F)
    P = WQ @ Kmat.T                       # (F, C)
    V = label_emb @ WV                    # (C, F)
    # P rearranged so chunk k of the contraction dim sits at cols [k*C,(k+1)*C).
    Pr = np.ascontiguousarray(
        P.reshape(KC, 128, C).transpose(1, 0, 2).reshape(128, KC * C)
    )
    # Append the softmax-denominator ones column.
    V_aug = np.ascontiguousarray(
        np.concatenate([V, np.ones((C, 1), np.float32)], axis=1)
    )
    return Pr, V_aug


def kernel(inputs, WQ, label_emb, WK, WV) -> np.ndarray:
    inputs = np.asarray(inputs, dtype=np.float32)
    WQ = np.asarray(WQ, dtype=np.float32)
    label_emb = np.asarray(label_emb, dtype=np.float32)
    WK = np.asarray(WK, dtype=np.float32)
    WV = np.asarray(WV, dtype=np.float32)

    # Host-side weight folding (weights only -- no activations touched).
    Pr, V_aug = _prep_weights(WQ, label_emb, WK, WV)

    nc = _get_nc()
    in_maps = []
    for b in range(N_CORES):
        in_maps.append(
            {
                "xT": np.ascontiguousarray(inputs[b].T),
                "Pr": Pr,
                "Vm": V_aug,
            }
        )

    res = bass_utils.run_bass_kernel_spmd(nc, in_maps, list(range(N_CORES)))
    out = np.stack([res.results[b]["out"] for b in range(N_CORES)], axis=0)
    return out



# revision 2
# speedup vs baseline: 1.3035x; 1.3035x over previous
"""Trainium2 Bass kernel for nn_Attention_46222438039802 — bf16 I/O version.

Reference computation:
    Q      = inputs @ WQ                    # (B,S,F)
    Kmat   = label_emb @ WK                 # (C,F)
    scores = Q @ Kmat^T                     # (B,S,C)
    A      = softmax(scores, axis=-1)
    V      = label_emb @ WV                 # (C,F)
    out    = A @ V                          # (B,S,F)

Algebraic rewrite: scores = inputs @ (WQ @ Kmat^T) = inputs @ P, P : (F,C).
Device computes  out = softmax(x @ P) @ V,  data-parallel (1 batch el/core).

The previous fp32 kernel was DMA-bound: 16 MiB/core (8 in + 8 out) at
~280 GB/s = 60 us.  This version moves x, P, V and the output in bf16
(8.4 MiB/core), targeting ~30 us.  Accumulation stays fp32 in PSUM and
softmax denominators stay fp32, so the only precision loss is bf16
rounding of the operands (~1e-3 rel, vs the 2e-2 gate).

Device layout per core (x = inputs[b], pre-transposed + bf16 on host):
  - xT (F, S) bf16 so the contraction dim F lies on SBUF partitions;
    loaded block-by-block (NB blocks of SB=S/NB rows) into a bufs=3 pool
    so loads prefetch across blocks and For_i iterations.
  - scoresT = P-chunks @ xT-chunks accumulated in PSUM as [C=64, SB].
  - exp on the Scalar engine straight out of PSUM (max-subtract skipped:
    scores ~ N(0,1)), output bf16.
  - expT [64, SB] is already the stationary-operand layout for A @ V:
    out_tile [128, F] = expT_tile^T @ V.  Softmax denominator via a
    ones-column appended to V (V_aug[:, F] == 1).  Zero transposes.
  - normalization fused into the PSUM->SBUF copy (scale = 1/denom),
    split across Scalar and Vector engines, output bf16.
"""

import ml_dtypes
import numpy as np

import concourse.bass as bass
import concourse.mybir as mybir
from concourse import bacc, bass_utils
from concourse.tile import TileContext

B, S, F, C = 8, 2048, 1024, 64
N_CORES = 8
FP32 = mybir.dt.float32
BF16 = mybir.dt.bfloat16

KC = F // 128            # 8 contraction chunks of 128


def _build_bass(n_iters: int = 1, variant: str = "bigstore",
                n_blocks: int = 4) -> bass.Bass:
    """Build the kernel; n_iters > 1 wraps the computation in a hardware
    For_i loop for wall-clock slope benchmarking (kernel() uses n_iters=1).
    variant: 'bigstore' | 'dma_only' (loads + stores, no compute).
    n_blocks: S-block pipelining granularity."""
    nc = bacc.Bacc()
    NB = n_blocks
    SB = S // NB             # rows per block
    NTB = SB // 128          # output s-tiles per block
    NMM = max(SB // 512, 1)  # scores matmuls per k-chunk (N<=512)
    NW = min(SB, 512)        # scores matmul moving width

    xT = nc.dram_tensor("xT", [F, S], BF16, kind="ExternalInput")
    Pr = nc.dram_tensor("Pr", [128, KC * C], BF16, kind="ExternalInput")
    Vm = nc.dram_tensor("Vm", [C, F + 1], BF16, kind="ExternalInput")
    out = nc.dram_tensor("out", [S, F], BF16, kind="ExternalOutput")

    with TileContext(nc) as tc:
        with (
            tc.tile_pool(name="consts", bufs=1) as consts,
            tc.tile_pool(name="xt", bufs=3) as xt_pool,
            tc.tile_pool(name="expT", bufs=2) as exp_pool,
            tc.tile_pool(name="recip", bufs=2) as recip_pool,
            tc.tile_pool(name="osb", bufs=3) as out_pool,
            tc.tile_pool(name="scps", bufs=2, space="PSUM") as sc_psum,
            tc.tile_pool(name="numps", bufs=2, space="PSUM") as num_psum,
            tc.tile_pool(name="denps", bufs=2, space="PSUM") as den_psum,
        ):
          def one_iter(_iv=None):
              def load_block(hh):
                  xt_b = xt_pool.tile([128, KC * SB], BF16, tag="xt")
                  nc.sync.dma_start(
                      xt_b[:, :].rearrange("p (k s) -> p k s", k=KC),
                      xT[:, hh * SB : (hh + 1) * SB].rearrange(
                          "(k p) s -> p k s", p=128
                      ),
                  )
                  return xt_b

              # First activation block DMA goes out ahead of the small
              # const loads so block-0 compute starts as early as possible.
              xt_blocks = [load_block(0)]
              P_sb = consts.tile([128, KC * C], BF16)
              nc.sync.dma_start(P_sb[:], Pr[:, :])
              V_sb = consts.tile([C, F + 1], BF16)
              nc.sync.dma_start(V_sb[:], Vm[:, :])
              for hh in range(1, NB):
                  xt_blocks.append(load_block(hh))

              if variant == "dma_only":
                  for h in range(NB):
                      dst = out[h * SB : (h + 1) * SB, :].rearrange(
                          "(t p) f -> p t f", p=128
                      )
                      srcv = xt_blocks[h][:, : NTB * F].rearrange(
                          "p (t f) -> p t f", f=F
                      )
                      nc.scalar.dma_start(dst, srcv)
                  return

              for h in range(NB):
                  xt_b = xt_blocks[h]
                  # scoresT[c, s] for this block, accumulated over F.
                  scT = sc_psum.tile([C, SB], FP32)
                  for k in range(KC):
                      for n in range(NMM):
                          nc.tensor.matmul(
                              scT[:, n * NW : (n + 1) * NW],
                              lhsT=P_sb[:, k * C : (k + 1) * C],
                              rhs=xt_b[:, k * SB + n * NW : k * SB + (n + 1) * NW],
                              start=(k == 0),
                              stop=(k == KC - 1),
                          )

                  expT = exp_pool.tile([C, SB], BF16)
                  nc.scalar.activation(
                      expT[:], scT[:], mybir.ActivationFunctionType.Exp
                  )

                  # Row-sums of exp via the ones-column of V_aug.
                  den = den_psum.tile([128, NTB], FP32)
                  for t in range(NTB):
                      nc.tensor.matmul(
                          den[:, t : t + 1],
                          lhsT=expT[:, t * 128 : (t + 1) * 128],
                          rhs=V_sb[:, F : F + 1],
                          start=True,
                          stop=True,
                      )
                  recip = recip_pool.tile([128, NTB], FP32)
                  nc.vector.reciprocal(recip[:], den[:])

                  osb_big = out_pool.tile([128, NTB * F], BF16, tag="osb")
                  for t in range(NTB):
                      num = num_psum.tile([128, F], FP32)
                      for n in range(F // 512):
                          nc.tensor.matmul(
                              num[:, n * 512 : (n + 1) * 512],
                              lhsT=expT[:, t * 128 : (t + 1) * 128],
                              rhs=V_sb[:, n * 512 : (n + 1) * 512],
                              start=True,
                              stop=True,
                          )
                      osb = osb_big[:, t * F : (t + 1) * F]
                      # Normalize while copying PSUM->SBUF, split across the
                      # Scalar and Vector engines.
                      nc.scalar.mul(osb[:, 0:512], num[:, 0:512], recip[:, t : t + 1])
                      nc.vector.tensor_scalar_mul(
                          osb[:, 512:1024], num[:, 512:1024], recip[:, t : t + 1]
                      )
                  row0 = h * SB
                  dst = out[row0 : row0 + SB, :].rearrange(
                      "(t p) f -> p t f", p=128
                  )
                  srcv = osb_big[:, :].rearrange("p (t f) -> p t f", f=F)
                  # Store on the Scalar engine's HWDGE ring so stores
                  # overlap the SP-ring input loads.
                  nc.scalar.dma_start(dst, srcv)

          if n_iters == 1:
              one_iter()
          else:
              with tc.For_i(0, n_iters, 1) as iv:
                  one_iter(iv)

    nc.compile()
    return nc


_NC_CACHE: list = []


def _get_nc() -> bass.Bass:
    if not _NC_CACHE:
        _NC_CACHE.append(_build_bass())
    return _NC_CACHE[0]


def _prep_weights(WQ, label_emb, WK, WV):
    Kmat = label_emb @ WK                 # (C, F)
    P = WQ @ Kmat.T                       # (F, C)
    V = label_emb @ WV                    # (C, F)
    # P rearranged so chunk k of the contraction dim sits at cols [k*C,(k+1)*C).
    Pr = np.ascontiguousarray(
        P.reshape(KC, 128, C).transpose(1, 0, 2).reshape(128, KC * C)
    ).astype(ml_dtypes.bfloat16)
    # Append the softmax-denominator ones column.
    V_aug = np.ascontiguousarray(
        np.concatenate([V, np.ones((C, 1), np.float32)], axis=1)
    ).astype(ml_dtypes.bfloat16)
    return Pr, V_aug


def _prep_x(inputs_b: np.ndarray) -> np.ndarray:
    return np.ascontiguousarray(inputs_b.T).astype(ml_dtypes.bfloat16)


def kernel(inputs, WQ, label_emb, WK, WV) -> np.ndarray:
    inputs = np.asarray(inputs, dtype=np.float32)
    WQ = np.asarray(WQ, dtype=np.float32)
    label_emb = np.asarray(label_emb, dtype=np.float32)
    WK = np.asarray(WK, dtype=np.float32)
    WV = np.asarray(WV, dtype=np.float32)

    # Host-side weight folding (weights only -- no activations touched).
    Pr, V_aug = _prep_weights(WQ, label_emb, WK, WV)

    nc = _get_nc()
    in_maps = []
    for b in range(N_CORES):
        in_maps.append({"xT": _prep_x(inputs[b]), "Pr": Pr, "Vm": V_aug})

    res = bass_utils.run_bass_kernel_spmd(nc, in_maps, list(range(N_CORES)))
    out = np.stack(
        [res.results[b]["out"].astype(np.float32) for b in range(N_CORES)],
        axis=0,
    )
    return out
